# revision 37
# baseline (speedup 1.0000x reference)
"""Causal linear attention (ELU+1 feature map) on 8 TRN2 NeuronCores.

Math (per batch b, head h):
    phi(x) = elu(x) + 1 = max(x+1, min(exp(x), 1))
    S_t = S_{t-1} + phi(k_t)^T v_t        (DxD state)
    z_t = z_{t-1} + phi(k_t)              (D normalizer)
    out_t = (phi(q_t) @ S_t) / (phi(q_t) . z_t + eps)

Sharding: B*H = 32 independent (b,h) pairs -> 4 per core, processed as
2 groups of 2 partition-packed pairs, emission-interleaved so the PE
always has work while each group's serial state chain advances.

Host marshalling (layout/dtype only + the affine "+1" pre-bias and the
final normalizer division):
  - q,k sent as y = (x+1) bf16; device computes phi = max(min(exp(y-1),1), y).
  - q pre-transposed per group to [128=2x64 d-rows, T]; k sent natural
    (chunked); the d-major phi(k) is produced on the PE via transpose-mode
    matmuls.  v is sent with a ones column ([t, 65]) so every matmul
    carries the normalizer for free.
  - device writes num|den [t, 65] bf16; host divides and unpermutes.

Pipeline: DMA and phi are issued per quarter-tile (4 chunks) so matmuls
start ~12us in; the per-chunk S snapshots are emitted as early chains
per group, decoupled from the num/A_T wave loop.  Matmuls are emitted
in homogeneous runs (A_T pairs, col-split intra, row-paired inter) so
consecutive MMs land in disjoint PE row/col groups and their LDWEIGHTS
pipeline — measured ~25-55ns/MM vs ~160ns for mixed-shape emission.

Measured on 8 axon trn2 cores: ~51.1us HW exec (baseline: 231.75us),
rel err 4.9e-3 vs the fp32 reference.
"""

import numpy as np
import ml_dtypes

import concourse.bass as bass
import concourse.tile as tile
from concourse import bacc, mybir
from concourse.bass_utils import run_bass_kernel_spmd

F32 = mybir.dt.float32
BF16 = mybir.dt.bfloat16
ALU = mybir.AluOpType
ACT = mybir.ActivationFunctionType

B, T, H, D = 2, 2048, 16, 64
PAIRS = B * H            # 32
NCORES = 8
PPC = PAIRS // NCORES    # 4 pairs per core
C = 128                  # chunk length
NCH = T // C             # 16 chunks
WAVE = 4                 # chunks per pn wave
HALF = NCH // 2          # 8 chunks per DMA/phi slab
DA = D + 1               # 65
GROUPS = PPC // 2        # 2 pairs per group

BF = ml_dtypes.bfloat16
_CACHE = {}


class _GroupCtx:
    pass


def _emit(ctx, tc, qtd, knd, vad, od):
    nc = tc.nc
    cpool = ctx.enter_context(tc.tile_pool(name="const", bufs=1))
    sb = ctx.enter_context(tc.tile_pool(name="sb", bufs=1))
    psum = ctx.enter_context(tc.tile_pool(name="psum", bufs=1, space="PSUM"))

    ones = cpool.tile([128, 128], BF16, tag="ones")
    nc.gpsimd.memset(ones[:, :], 1.0)
    mask = cpool.tile([128, 128], BF16, tag="mask")
    nc.gpsimd.affine_select(
        mask[:, :], ones[:, :], pattern=[[1, 128]], base=0,
        channel_multiplier=-1, compare_op=ALU.is_ge, fill=0.0)
    masks4 = mask[:, :].unsqueeze(1).broadcast_to([128, WAVE, 128])
    ident = cpool.tile([128, 128], BF16, tag="ident")
    nc.gpsimd.affine_select(
        ident[:, :], ones[:, :], pattern=[[-1, 128]], base=0,
        channel_multiplier=1, compare_op=ALU.is_equal, fill=0.0)
    neg1 = cpool.tile([128, 1], F32, tag="neg1")
    nc.gpsimd.memset(neg1[:, :], -1.0)

    G = []
    for g in range(GROUPS):
        gc = _GroupCtx()
        gc.qtr = sb.tile([128, T], BF16, tag=f"qtr{g}", name=f"qtr{g}")
        gc.knr = sb.tile([128, T], BF16, tag=f"knr{g}", name=f"knr{g}")
        gc.va = sb.tile([128, 2 * NCH * DA], BF16, tag=f"va{g}", name=f"va{g}")
        gc.qt = sb.tile([128, T], BF16, tag=f"qt{g}", name=f"qt{g}")
        gc.kn = sb.tile([128, T], BF16, tag=f"kn{g}", name=f"kn{g}")
        gc.kt = sb.tile([128, NCH * 128], BF16, tag=f"kt{g}", name=f"kt{g}")
        gc.osb = sb.tile([128, 2 * NCH * DA], BF16, tag=f"osb{g}", name=f"osb{g}")
        gc.pS = psum.tile([128, 512], F32, tag=f"pS{g}", bufs=1,
                          name=f"pS{g}")[:, 0:DA]
        gc.ssb = [None] * NCH
        G.append(gc)

    # ---- input DMAs, half-tile granularity, kn first ----------------------
    def dma_part(g, q0, q1):
        # load chunks [q0, q1) of all three tensors
        gc = G[g]
        sl = slice(q0 * C, q1 * C)
        nc.sync.dma_start(gc.knr[:, sl],
                      knd[g].rearrange("p c r d -> p (c r d)")[:, sl])
        nc.sync.dma_start(gc.qtr[:, sl], qtd[g][:, sl])
        va3 = gc.va[:, :].rearrange("p (r c d) -> p r c d", r=2, d=DA)
        nc.sync.dma_start(va3[:, :, q0:q1, :], vad[g][:, :, q0:q1, :])

    # ---- phi + kt transposes + state chain per (g, half) ------------------
    def phi_part(g, c0, c1):
        gc = G[g]
        n = (c1 - c0) * C
        for idx, (srct, dstt) in enumerate(((gc.knr, gc.kn), (gc.qtr, gc.qt))):
            ap_s = srct[:, c0 * C:c1 * C]
            ap_d = dstt[:, c0 * C:c1 * C]
            e = sb.tile([128, HALF * C], BF16, tag="phie", bufs=4,
                        name=f"e{g}_{c0}_{idx}")
            nc.scalar.activation(e[:, 0:n], ap_s, ACT.Exp, bias=neg1[:, :])
            nc.vector.scalar_tensor_tensor(
                ap_d, e[:, 0:n], 1.0, ap_s, ALU.min, ALU.max)

    def kt_part(g, c0, c1):
        gc = G[g]
        n = c1 - c0
        pt = psum.tile([128, HALF * 128], BF16, tag="pt", bufs=1,
                       name=f"pt{g}_{c0}")
        for cc in range(n):
            c = c0 + cc
            nc.tensor.matmul(
                pt[:, cc * 128:(cc + 1) * 128],
                gc.kn[:, c * 128:(c + 1) * 128], ident[:, :],
                is_transpose=True,
                start=(cc == 0), stop=(cc == n - 1),
                skip_group_check=True)
        nc.vector.tensor_copy(
            gc.kt[:, c0 * 128:c1 * 128], pt[:, 0:n * 128])

    def state_chain(g, c0, c1):
        gc = G[g]
        for c in range(c0, c1):
            for pi in range(2):
                nc.tensor.matmul(
                    gc.pS[pi * 64:(pi + 1) * 64, :],
                    gc.kn[:, c * 128 + pi * 64: c * 128 + (pi + 1) * 64],
                    gc.va[:, pi * NCH * DA + c * DA: pi * NCH * DA + (c + 1) * DA],
                    start=(c == 0), stop=(c == NCH - 1),
                    skip_group_check=True)
            if c < NCH - 1:
                s = sb.tile([128, DA], BF16, tag=f"ssb{g}", bufs=NCH,
                            name=f"ssb{g}_{c}")
                if c % 2 == 0:
                    nc.scalar.copy(s[:, :], gc.pS[:, :])
                else:
                    nc.vector.tensor_copy(s[:, :], gc.pS[:, :])
                gc.ssb[c] = s

    # ---- A slab (8 chunks) + pn waves (4 chunks) --------------------------
    def a_wave(g, w):
        gc = G[g]
        gc.aw = []
        for pi in range(2):
            pA = psum.tile([128, WAVE * 128], F32, tag=f"pA{pi}", bufs=1,
                           name=f"pA{g}_{w}_{pi}")
            for cc in range(WAVE):
                c = w * WAVE + cc
                nc.tensor.matmul(
                    pA[:, cc * 128:(cc + 1) * 128],
                    gc.kt[pi * 64:(pi + 1) * 64, c * 128:(c + 1) * 128],
                    gc.qt[pi * 64:(pi + 1) * 64, c * 128:(c + 1) * 128],
                    start=(cc == 0), stop=(cc == WAVE - 1),
                    skip_group_check=True,
                    tile_position=(pi * 64, 0))
            a = sb.tile([128, WAVE * 128], BF16, tag=f"aw{pi}", bufs=2,
                        name=f"aw{g}_{w}_{pi}")
            nc.vector.tensor_tensor(
                a[:, :].rearrange("p (c f) -> p c f", f=128),
                pA[:, :].rearrange("p (c f) -> p c f", f=128),
                masks4, ALU.mult)
            gc.aw.append(a)

    def pn_wave(g, w):
        gc = G[g]
        pn = [psum.tile([128, 512], F32, tag=f"pn{pi}", bufs=1,
                        name=f"pn{g}_{w}_{pi}")[:, 0:WAVE * DA]
              for pi in range(2)]
        # run 1: intra, col-split into i-halves (col-group pairs)
        for cc in range(WAVE):
            c = w * WAVE + cc
            for pi in range(2):
                for ih in range(2):
                    nc.tensor.matmul(
                        pn[pi][ih * 64:(ih + 1) * 64, cc * DA:(cc + 1) * DA],
                        gc.aw[pi][:, cc * 128 + ih * 64: cc * 128 + (ih + 1) * 64],
                        gc.va[:,
                              pi * NCH * DA + c * DA: pi * NCH * DA + (c + 1) * DA],
                        start=(cc == 0), stop=False,
                        skip_group_check=True,
                        tile_position=(0, ih * 64))
        # run 2: inter, row-group pairs
        for cc in range(WAVE):
            c = w * WAVE + cc
            for pi in range(2):
                if c > 0:
                    nc.tensor.matmul(
                        pn[pi][:, cc * DA:(cc + 1) * DA],
                        gc.qt[pi * 64:(pi + 1) * 64, c * 128:(c + 1) * 128],
                        gc.ssb[c - 1][pi * 64:(pi + 1) * 64, :],
                        start=False, stop=(cc == WAVE - 1),
                        skip_group_check=True,
                        tile_position=(pi * 64, 0))
                else:
                    # c == 0: no inter; re-close the group on a zero-impact mm
                    # (never reached: c==0 only at w==0, cc==0, stop comes from
                    # the cc==3 inter)
                    pass
        for pi in range(2):
            nc.scalar.activation(
                gc.osb[:, pi * NCH * DA + w * WAVE * DA:
                       pi * NCH * DA + (w + 1) * WAVE * DA],
                pn[pi][:, :], ACT.Copy)

    def out_dma(g, c0, c1):
        gc = G[g]
        for pi in range(2):
            nc.sync.dma_start(
                od[2 * g + pi][:, c0:c1, :]
                .rearrange("p c d -> p (c d)"),
                gc.osb[:, pi * NCH * DA + c0 * DA:
                       pi * NCH * DA + c1 * DA])

    # ---- global emission order -------------------------------------------
    Q = WAVE  # quarter = 4 chunks
    for g in range(GROUPS):
        dma_part(g, 0, Q)
    for g in range(GROUPS):
        dma_part(g, Q, HALF)
    for g in range(GROUPS):
        dma_part(g, HALF, NCH)
    for g in range(GROUPS):
        phi_part(g, 0, Q)
        kt_part(g, 0, Q)
        state_chain(g, 0, Q)
    for g in range(GROUPS):
        a_wave(g, 0)
    for g in range(GROUPS):
        phi_part(g, Q, HALF)
        kt_part(g, Q, HALF)
        state_chain(g, Q, HALF)
    for g in range(GROUPS):
        pn_wave(g, 0)
    for g in range(GROUPS):
        a_wave(g, 1)
    for g in range(GROUPS):
        pn_wave(g, 1)
    for g in range(GROUPS):
        phi_part(g, HALF, NCH)
        kt_part(g, HALF, 3 * Q)
        state_chain(g, HALF, 3 * Q)
    for g in range(GROUPS):
        out_dma(g, 0, HALF)
    for g in range(GROUPS):
        a_wave(g, 2)
    for g in range(GROUPS):
        pn_wave(g, 2)
    for g in range(GROUPS):
        kt_part(g, 3 * Q, NCH)
        state_chain(g, 3 * Q, NCH)
    for g in range(GROUPS):
        a_wave(g, 3)
    for g in range(GROUPS):
        out_dma(g, HALF, 3 * Q)
    for g in range(GROUPS):
        pn_wave(g, 3)
    for g in range(GROUPS):
        out_dma(g, 3 * Q, NCH)


def build_program():
    from contextlib import ExitStack

    nc = bacc.Bacc("TRN2", target_bir_lowering=False, debug=False,
                   num_devices=NCORES)
    qtd = nc.dram_tensor("qt", [GROUPS, 128, T], BF16, kind="ExternalInput").ap()
    knd = nc.dram_tensor("kn", [GROUPS, 128, NCH, 2, D], BF16,
                         kind="ExternalInput").ap()
    vad = nc.dram_tensor("va", [GROUPS, 128, 2, NCH, DA], BF16,
                         kind="ExternalInput").ap()
    od = nc.dram_tensor("out", [PPC, 128, NCH, DA], BF16,
                        kind="ExternalOutput").ap()
    with tile.TileContext(nc) as tc:
        with ExitStack() as ctx:
            _emit(ctx, tc, qtd, knd, vad, od)
    nc.compile()
    return nc


def _to_pairs(x):
    # [B, T, H, D] -> [PAIRS, T, D]
    return np.ascontiguousarray(np.transpose(x, (0, 2, 1, 3))).reshape(PAIRS, T, D)


def _to_chunked(x):
    # [PAIRS, T, D'] -> [PAIRS, i=128, c=16, D']  with t = c*128 + i
    d = x.shape[-1]
    x = x.reshape(PAIRS, NCH, C, d)
    return np.ascontiguousarray(np.transpose(x, (0, 2, 1, 3)))


def _marshal(q, k, v):
    yq = _to_pairs(np.asarray(q)).astype(BF) + np.asarray(1.0, dtype=BF)
    yk = _to_pairs(np.asarray(k)).astype(BF) + np.asarray(1.0, dtype=BF)
    vv = _to_pairs(np.asarray(v)).astype(BF)

    # qt: [PAIRS, D, T] -> per-core groups [PPC//2, 128, T]
    qt = np.ascontiguousarray(np.transpose(yq, (0, 2, 1)))
    qt = qt.reshape(PAIRS // 2, 2 * D, T)                        # group-packed
    kn = _to_chunked(yk)                                         # [P,128,16,64]
    kn = np.ascontiguousarray(
        np.transpose(kn.reshape(PAIRS // 2, 2, 128, NCH, D), (0, 2, 3, 1, 4)))
    ones = np.ones((PAIRS, T, 1), dtype=BF)
    va = _to_chunked(np.concatenate([vv, ones], axis=-1))        # [P,128,16,65]
    va = np.ascontiguousarray(
        np.transpose(va.reshape(PAIRS // 2, 2, 128, NCH, DA), (0, 2, 1, 3, 4)))
    return qt, kn, va


def kernel(q, k, v, trace=False):
    if "nc" not in _CACHE:
        _CACHE["nc"] = build_program()
    nc = _CACHE["nc"]

    qt, kn, va = _marshal(q, k, v)
    gpc = GROUPS  # groups per core

    in_maps = []
    for core in range(NCORES):
        sl = slice(core * gpc, (core + 1) * gpc)
        in_maps.append({
            "qt": np.ascontiguousarray(qt[sl]),
            "kn": np.ascontiguousarray(kn[sl]),
            "va": np.ascontiguousarray(va[sl]),
        })

    res = run_bass_kernel_spmd(nc, in_maps, core_ids=list(range(NCORES)),
                               trace=trace)
    _CACHE["last_result"] = res
    outs = np.concatenate([np.asarray(r["out"]) for r in res.results], axis=0)

    outs = outs.astype(np.float32)                               # [P,128,16,65]
    num = outs[..., 0:D]
    den = outs[..., D:DA] + 1e-6
    o = num / den                                                # [P,128,16,64]
    o = np.transpose(o, (0, 2, 1, 3)).reshape(B, H, T, D)
    return np.ascontiguousarray(np.transpose(o, (0, 2, 1, 3)))


# revision 38
# speedup vs baseline: 1.1864x; 1.1864x over previous
"""Causal linear attention (ELU+1 feature map) on 8 TRN2 NeuronCores.

Math (per batch b, head h):
    phi(x) = elu(x) + 1 = max(x+1, min(exp(x), 1))
    S_t = S_{t-1} + phi(k_t)^T v_t        (DxD state)
    z_t = z_{t-1} + phi(k_t)              (D normalizer)
    out_t = (phi(q_t) @ S_t) / (phi(q_t) . z_t + eps)

Sharding: B*H = 32 independent (b,h) pairs -> 4 per core, processed as
2 groups of 2 partition-packed pairs, emission-interleaved so the PE
always has work while each group's serial state chain advances.

Host marshalling (layout/dtype only + the affine "+1" pre-bias and the
final normalizer division):
  - q,k sent as y = (x+1) bf16; device computes phi = max(min(exp(y-1),1), y).
  - q pre-transposed per group to [128=2x64 d-rows, T]; k sent natural
    (chunked); the d-major phi(k) is produced on the PE via transpose-mode
    matmuls.  v is sent with a ones column ([t, 65]) so every matmul
    carries the normalizer for free.
  - device writes num|den [t, 65] bf16; host divides and unpermutes.

Pipeline: DMA and phi are issued per quarter-tile (4 chunks) so matmuls
start ~12us in; the per-chunk S snapshots are emitted as early chains
per group, decoupled from the num/A_T wave loop.  Matmuls are emitted
in homogeneous runs (A_T pairs, col-split intra, row-paired inter) so
consecutive MMs land in disjoint PE row/col groups and their LDWEIGHTS
pipeline — measured ~25-55ns/MM vs ~160ns for mixed-shape emission.

Measured on 8 axon trn2 cores: ~51.1us HW exec (baseline: 231.75us),
rel err 4.9e-3 vs the fp32 reference.
"""

import numpy as np
import ml_dtypes

import concourse.bass as bass
import concourse.tile as tile
from concourse import bacc, mybir
from concourse.bass_utils import run_bass_kernel_spmd

F32 = mybir.dt.float32
BF16 = mybir.dt.bfloat16
ALU = mybir.AluOpType
ACT = mybir.ActivationFunctionType

B, T, H, D = 2, 2048, 16, 64
PAIRS = B * H            # 32
NCORES = 8
PPC = PAIRS // NCORES    # 4 pairs per core
C = 128                  # chunk length
NCH = T // C             # 16 chunks
WAVE = 4                 # chunks per pn wave
HALF = NCH // 2          # 8 chunks per DMA/phi slab
DA = D + 1               # 65
GROUPS = PPC // 2        # 2 pairs per group

BF = ml_dtypes.bfloat16
_CACHE = {}


class _GroupCtx:
    pass


def _emit(ctx, tc, qtd, knd, vad, od):
    nc = tc.nc
    cpool = ctx.enter_context(tc.tile_pool(name="const", bufs=1))
    sb = ctx.enter_context(tc.tile_pool(name="sb", bufs=1))
    psum = ctx.enter_context(tc.tile_pool(name="psum", bufs=1, space="PSUM"))

    ones = cpool.tile([128, 128], BF16, tag="ones")
    nc.gpsimd.memset(ones[:, :], 1.0)
    mask = cpool.tile([128, 128], BF16, tag="mask")
    nc.gpsimd.affine_select(
        mask[:, :], ones[:, :], pattern=[[1, 128]], base=0,
        channel_multiplier=-1, compare_op=ALU.is_ge, fill=0.0)
    masks4 = mask[:, :].unsqueeze(1).broadcast_to([128, WAVE, 128])
    ident = cpool.tile([128, 128], BF16, tag="ident")
    nc.gpsimd.affine_select(
        ident[:, :], ones[:, :], pattern=[[-1, 128]], base=0,
        channel_multiplier=1, compare_op=ALU.is_equal, fill=0.0)
    neg1 = cpool.tile([128, 1], F32, tag="neg1")
    nc.gpsimd.memset(neg1[:, :], -1.0)

    G = []
    for g in range(GROUPS):
        gc = _GroupCtx()
        gc.qtr = sb.tile([128, T], BF16, tag=f"qtr{g}", name=f"qtr{g}")
        gc.knr = sb.tile([128, T], BF16, tag=f"knr{g}", name=f"knr{g}")
        gc.va = sb.tile([128, 2 * NCH * DA], BF16, tag=f"va{g}", name=f"va{g}")
        gc.qt = sb.tile([128, T], BF16, tag=f"qt{g}", name=f"qt{g}")
        gc.kn = sb.tile([128, T], BF16, tag=f"kn{g}", name=f"kn{g}")
        gc.kt = sb.tile([128, NCH * 128], BF16, tag=f"kt{g}", name=f"kt{g}")
        gc.osb = sb.tile([128, 2 * NCH * DA], BF16, tag=f"osb{g}", name=f"osb{g}")
        gc.pS = psum.tile([128, 512], F32, tag=f"pS{g}", bufs=1,
                          name=f"pS{g}")[:, 0:DA]
        gc.ssb = [None] * NCH
        G.append(gc)

    # ---- input DMAs, half-tile granularity, kn first ----------------------
    def dma_part(g, q0, q1):
        # load chunks [q0, q1) of all three tensors
        gc = G[g]
        sl = slice(q0 * C, q1 * C)
        nc.sync.dma_start(gc.knr[:, sl],
                      knd[g].rearrange("p c r d -> p (c r d)")[:, sl])
        nc.sync.dma_start(gc.qtr[:, sl], qtd[g][:, sl])
        va3 = gc.va[:, :].rearrange("p (r c d) -> p r c d", r=2, d=DA)
        nc.sync.dma_start(va3[:, :, q0:q1, :], vad[g][:, :, q0:q1, :])

    # ---- phi + kt transposes + state chain per (g, half) ------------------
    def phi_part(g, c0, c1):
        gc = G[g]
        n = (c1 - c0) * C
        for idx, (srct, dstt) in enumerate(((gc.knr, gc.kn), (gc.qtr, gc.qt))):
            ap_s = srct[:, c0 * C:c1 * C]
            ap_d = dstt[:, c0 * C:c1 * C]
            e = sb.tile([128, HALF * C], BF16, tag="phie", bufs=4,
                        name=f"e{g}_{c0}_{idx}")
            nc.scalar.activation(e[:, 0:n], ap_s, ACT.Exp, bias=neg1[:, :])
            nc.vector.scalar_tensor_tensor(
                ap_d, e[:, 0:n], 1.0, ap_s, ALU.min, ALU.max)

    def kt_part(g, c0, c1):
        gc = G[g]
        n = c1 - c0
        pt = psum.tile([128, HALF * 128], BF16, tag="pt", bufs=1,
                       name=f"pt{g}_{c0}")
        for cc in range(n):
            c = c0 + cc
            nc.tensor.matmul(
                pt[:, cc * 128:(cc + 1) * 128],
                gc.kn[:, c * 128:(c + 1) * 128], ident[:, :],
                is_transpose=True,
                start=(cc == 0), stop=(cc == n - 1),
                skip_group_check=True)
        nc.vector.tensor_copy(
            gc.kt[:, c0 * 128:c1 * 128], pt[:, 0:n * 128])

    def state_chain(g, c0, c1):
        gc = G[g]
        for c in range(c0, c1):
            for pi in range(2):
                nc.tensor.matmul(
                    gc.pS[pi * 64:(pi + 1) * 64, :],
                    gc.kn[:, c * 128 + pi * 64: c * 128 + (pi + 1) * 64],
                    gc.va[:, pi * NCH * DA + c * DA: pi * NCH * DA + (c + 1) * DA],
                    start=(c == 0), stop=(c == NCH - 1),
                    skip_group_check=True)
            if c < NCH - 1:
                s = sb.tile([128, DA], BF16, tag=f"ssb{g}", bufs=NCH,
                            name=f"ssb{g}_{c}")
                if c % 2 == 0:
                    nc.scalar.copy(s[:, :], gc.pS[:, :])
                else:
                    nc.vector.tensor_copy(s[:, :], gc.pS[:, :])
                gc.ssb[c] = s

    # ---- A slab (8 chunks) + pn waves (4 chunks) --------------------------
    def a_wave(g, w):
        gc = G[g]
        gc.aw = []
        for pi in range(2):
            pA = psum.tile([128, WAVE * 128], F32, tag=f"pA{pi}", bufs=1,
                           name=f"pA{g}_{w}_{pi}")
            for cc in range(WAVE):
                c = w * WAVE + cc
                nc.tensor.matmul(
                    pA[:, cc * 128:(cc + 1) * 128],
                    gc.kt[pi * 64:(pi + 1) * 64, c * 128:(c + 1) * 128],
                    gc.qt[pi * 64:(pi + 1) * 64, c * 128:(c + 1) * 128],
                    start=(cc == 0), stop=(cc == WAVE - 1),
                    skip_group_check=True,
                    tile_position=(pi * 64, 0))
            a = sb.tile([128, WAVE * 128], BF16, tag=f"aw{pi}", bufs=2,
                        name=f"aw{g}_{w}_{pi}")
            nc.vector.tensor_tensor(
                a[:, :].rearrange("p (c f) -> p c f", f=128),
                pA[:, :].rearrange("p (c f) -> p c f", f=128),
                masks4, ALU.mult)
            gc.aw.append(a)

    def pn_wave(g, w):
        gc = G[g]
        pn = [psum.tile([128, 512], F32, tag=f"pn{pi}", bufs=1,
                        name=f"pn{g}_{w}_{pi}")[:, 0:WAVE * DA]
              for pi in range(2)]
        # run 1: intra, col-split into i-halves (col-group pairs)
        for pi in range(2):
            for cc in range(WAVE):
                c = w * WAVE + cc
                for ih in range(2):
                    nc.tensor.matmul(
                        pn[pi][ih * 64:(ih + 1) * 64, cc * DA:(cc + 1) * DA],
                        gc.aw[pi][:, cc * 128 + ih * 64: cc * 128 + (ih + 1) * 64],
                        gc.va[:,
                              pi * NCH * DA + c * DA: pi * NCH * DA + (c + 1) * DA],
                        start=(cc == 0), stop=False,
                        skip_group_check=True,
                        tile_position=(0, ih * 64))
        # run 2: inter, row-group pairs
        for cc in range(WAVE):
            c = w * WAVE + cc
            for pi in range(2):
                if c > 0:
                    nc.tensor.matmul(
                        pn[pi][:, cc * DA:(cc + 1) * DA],
                        gc.qt[pi * 64:(pi + 1) * 64, c * 128:(c + 1) * 128],
                        gc.ssb[c - 1][pi * 64:(pi + 1) * 64, :],
                        start=False, stop=(cc == WAVE - 1),
                        skip_group_check=True,
                        tile_position=(pi * 64, 0))
                else:
                    # c == 0: no inter; re-close the group on a zero-impact mm
                    # (never reached: c==0 only at w==0, cc==0, stop comes from
                    # the cc==3 inter)
                    pass
        for pi in range(2):
            nc.scalar.activation(
                gc.osb[:, pi * NCH * DA + w * WAVE * DA:
                       pi * NCH * DA + (w + 1) * WAVE * DA],
                pn[pi][:, :], ACT.Copy)

    def out_dma(g, c0, c1):
        gc = G[g]
        for pi in range(2):
            nc.sync.dma_start(
                od[2 * g + pi][:, c0:c1, :]
                .rearrange("p c d -> p (c d)"),
                gc.osb[:, pi * NCH * DA + c0 * DA:
                       pi * NCH * DA + c1 * DA])

    # ---- global emission order -------------------------------------------
    Q = WAVE  # quarter = 4 chunks
    for g in range(GROUPS):
        dma_part(g, 0, Q)
    for g in range(GROUPS):
        dma_part(g, Q, HALF)
    for g in range(GROUPS):
        dma_part(g, HALF, NCH)
    for g in range(GROUPS):
        phi_part(g, 0, Q)
        kt_part(g, 0, Q)
        state_chain(g, 0, Q)
    for g in range(GROUPS):
        a_wave(g, 0)
    for g in range(GROUPS):
        phi_part(g, Q, HALF)
        kt_part(g, Q, HALF)
        state_chain(g, Q, HALF)
    for g in range(GROUPS):
        pn_wave(g, 0)
    for g in range(GROUPS):
        a_wave(g, 1)
    for g in range(GROUPS):
        pn_wave(g, 1)
    for g in range(GROUPS):
        phi_part(g, HALF, NCH)
        kt_part(g, HALF, 3 * Q)
        state_chain(g, HALF, 3 * Q)
    for g in range(GROUPS):
        out_dma(g, 0, HALF)
    for g in range(GROUPS):
        a_wave(g, 2)
    for g in range(GROUPS):
        pn_wave(g, 2)
    for g in range(GROUPS):
        kt_part(g, 3 * Q, NCH)
        state_chain(g, 3 * Q, NCH)
    for g in range(GROUPS):
        a_wave(g, 3)
    for g in range(GROUPS):
        out_dma(g, HALF, 3 * Q)
    for g in range(GROUPS):
        pn_wave(g, 3)
    for g in range(GROUPS):
        out_dma(g, 3 * Q, NCH)


def build_program():
    from contextlib import ExitStack

    nc = bacc.Bacc("TRN2", target_bir_lowering=False, debug=False,
                   num_devices=NCORES)
    qtd = nc.dram_tensor("qt", [GROUPS, 128, T], BF16, kind="ExternalInput").ap()
    knd = nc.dram_tensor("kn", [GROUPS, 128, NCH, 2, D], BF16,
                         kind="ExternalInput").ap()
    vad = nc.dram_tensor("va", [GROUPS, 128, 2, NCH, DA], BF16,
                         kind="ExternalInput").ap()
    od = nc.dram_tensor("out", [PPC, 128, NCH, DA], BF16,
                        kind="ExternalOutput").ap()
    with tile.TileContext(nc) as tc:
        with ExitStack() as ctx:
            _emit(ctx, tc, qtd, knd, vad, od)
    nc.compile()
    return nc


def _to_pairs(x):
    # [B, T, H, D] -> [PAIRS, T, D]
    return np.ascontiguousarray(np.transpose(x, (0, 2, 1, 3))).reshape(PAIRS, T, D)


def _to_chunked(x):
    # [PAIRS, T, D'] -> [PAIRS, i=128, c=16, D']  with t = c*128 + i
    d = x.shape[-1]
    x = x.reshape(PAIRS, NCH, C, d)
    return np.ascontiguousarray(np.transpose(x, (0, 2, 1, 3)))


def _marshal(q, k, v):
    yq = _to_pairs(np.asarray(q)).astype(BF) + np.asarray(1.0, dtype=BF)
    yk = _to_pairs(np.asarray(k)).astype(BF) + np.asarray(1.0, dtype=BF)
    vv = _to_pairs(np.asarray(v)).astype(BF)

    # qt: [PAIRS, D, T] -> per-core groups [PPC//2, 128, T]
    qt = np.ascontiguousarray(np.transpose(yq, (0, 2, 1)))
    qt = qt.reshape(PAIRS // 2, 2 * D, T)                        # group-packed
    kn = _to_chunked(yk)                                         # [P,128,16,64]
    kn = np.ascontiguousarray(
        np.transpose(kn.reshape(PAIRS // 2, 2, 128, NCH, D), (0, 2, 3, 1, 4)))
    ones = np.ones((PAIRS, T, 1), dtype=BF)
    va = _to_chunked(np.concatenate([vv, ones], axis=-1))        # [P,128,16,65]
    va = np.ascontiguousarray(
        np.transpose(va.reshape(PAIRS // 2, 2, 128, NCH, DA), (0, 2, 1, 3, 4)))
    return qt, kn, va


def kernel(q, k, v, trace=False):
    if "nc" not in _CACHE:
        _CACHE["nc"] = build_program()
    nc = _CACHE["nc"]

    qt, kn, va = _marshal(q, k, v)
    gpc = GROUPS  # groups per core

    in_maps = []
    for core in range(NCORES):
        sl = slice(core * gpc, (core + 1) * gpc)
        in_maps.append({
            "qt": np.ascontiguousarray(qt[sl]),
            "kn": np.ascontiguousarray(kn[sl]),
            "va": np.ascontiguousarray(va[sl]),
        })

    res = run_bass_kernel_spmd(nc, in_maps, core_ids=list(range(NCORES)),
                               trace=trace)
    _CACHE["last_result"] = res
    outs = np.concatenate([np.asarray(r["out"]) for r in res.results], axis=0)

    outs = outs.astype(np.float32)                               # [P,128,16,65]
    num = outs[..., 0:D]
    den = outs[..., D:DA] + 1e-6
    o = num / den                                                # [P,128,16,64]
    o = np.transpose(o, (0, 2, 1, 3)).reshape(B, H, T, D)
    return np.ascontiguousarray(np.transpose(o, (0, 2, 1, 3)))
